# revision 32
# baseline (speedup 1.0000x reference)
"""Trainium2 Bass kernel for nn_MultiHeadAttention_77232101917088.

Causal MHA where only the LAST token's projected output is returned:
    out = (softmax_causal(q k^T / sqrt(hd)) v)[:, -1, :] @ Wo + bo

Only the last query row survives, so the problem collapses (the last
causal row attends to every position):
    q[b,:]        = x[b,-1,:] @ Wq
    u[b,h,d]      = sum_e Wk[d, h*128+e] * q[b, h*128+e]
    scores[b,h,j] = sum_d x[b,j,d] * u[b,h,d]           (no K/V materialized)
    p             = softmax_j(scores * 1/sqrt(hd))
    w[b,h,d]      = sum_j p[b,h,j] * x[b,j,d]
    ctx[b, hs]    = w[b,h,:] @ Wv[:, hs]
    out           = ctx @ Wo + bo

Sharding: ZERO collectives (first-collective init costs ~74us wall on
this stack).  Each core owns one batch and 4 heads (b = core//4, head
group = core%4), computing its 4 (b,h) pairs end to end from full-depth
x[b] in both layouts; the host sums the 4 output partials per batch.

Schedule: the two bulk rings (scalar HWDGE / gpsimd SWDGE) stream
[weights, xT half, xn half, late weight] so the scores pipeline starts
as soon as its half arrives; the sync ring carries only tiny inputs and
the DRAM-bounce transposes (u/scores/w -> transposed layouts via xbar
transpose-DMA), so its completion lanes never wait on bulk traffic.
Scores -> exp -> w are pipelined per j-half; ctx accumulates per d-half
behind the w bounce.  The softmax z comes from exp-with-accum_out on
the scalar engine (off the tensor queue); 1/sqrt(hd) is folded into
exp's scale; max-subtraction is skipped (|scores*ISCALE| < ~5 here).
"""

import numpy as np
from ml_dtypes import bfloat16

import concourse.bacc as bacc
import concourse.bass as bass
import concourse.mybir as mybir
import concourse.tile as tile
from concourse.bass_utils import run_bass_kernel_spmd

P = 128          # partitions
B = 2            # batch
S = 2048         # sequence length
D = 2048         # model dim
NH = 16          # heads
HD = 128         # head dim
NC = 8           # cores
HPC = 4          # heads per core
HW = HPC * HD    # per-core head-column width (512)
DT = D // P      # depth subtiles (16)
JT = S // P      # sequence subtiles (16)
NJC = 4          # 512-wide chunks for streaming matmuls
JC = S // NJC    # 512
HJ = S // 2      # j-half width (1024)
HT = JT // 2     # subtiles per half (8)
ISCALE = 1.0 / np.sqrt(HD)

FP32 = mybir.dt.float32
BF16 = mybir.dt.bfloat16


def _build_program():
    nc = bacc.Bacc(
        "TRN2",
        target_bir_lowering=False,
        debug=False,
        enable_asserts=False,
        num_devices=NC,
    )

    # ---- per-core DRAM inputs (host pre-arranged, contiguous loads) ------
    xlastT = nc.dram_tensor("xlastT", [P, DT], BF16, kind="ExternalInput").ap()
    ident = nc.dram_tensor("ident", [HPC, HPC], BF16, kind="ExternalInput").ap()
    wq = nc.dram_tensor("wq", [P, DT, HW], BF16, kind="ExternalInput").ap()
    wkT = nc.dram_tensor("wkT", [P, HPC, D], BF16, kind="ExternalInput").ap()
    xTa = nc.dram_tensor("xTa", [P, DT, HJ], BF16, kind="ExternalInput").ap()
    xTb = nc.dram_tensor("xTb", [P, DT, HJ], BF16, kind="ExternalInput").ap()
    xna = nc.dram_tensor("xna", [P, HT, D], BF16, kind="ExternalInput").ap()
    xnb = nc.dram_tensor("xnb", [P, HT, D], BF16, kind="ExternalInput").ap()
    wv = nc.dram_tensor("wv", [P, DT, HW], BF16, kind="ExternalInput").ap()
    wo = nc.dram_tensor("wo", [P, HPC, D], BF16, kind="ExternalInput").ap()
    bo_sh = nc.dram_tensor("bo_sh", [D], FP32, kind="ExternalInput").ap()

    out_sh = nc.dram_tensor("out_sh", [1, D], FP32, kind="ExternalOutput").ap()

    with tile.TileContext(nc) as tc:
        with (
            tc.tile_pool(name="persist", bufs=1) as pp,
            tc.tile_pool(name="work", bufs=1) as wp,
            tc.tile_pool(name="psA", bufs=2, space="PSUM") as psA,
            tc.tile_pool(name="psW", bufs=1, space="PSUM") as psW,
            tc.tile_pool(name="psB", bufs=2, space="PSUM") as psB,
            tc.tile_pool(name="dram", bufs=1, space="DRAM") as dp,
        ):
            # ---- loads -------------------------------------------------
            xlastT_sb = pp.tile([P, DT], BF16, name="xlastT_sb")
            nc.sync.dma_start(xlastT_sb[:], xlastT)
            ident_sb = pp.tile([HPC, HPC], BF16, name="ident_sb")
            nc.sync.dma_start(ident_sb[:], ident)
            bo_sb = pp.tile([1, D], FP32, name="bo_sb")
            nc.sync.dma_start(bo_sb[:], bo_sh.rearrange("(o m) -> o m", o=1))

            wq_sb = pp.tile([P, DT, HW], BF16, name="wq_sb")
            nc.scalar.dma_start(wq_sb[:], wq)
            wkT_sb = pp.tile([P, HPC, D], BF16, name="wkT_sb")
            nc.gpsimd.dma_start(wkT_sb[:], wkT)
            xTa_sb = pp.tile([P, DT, HJ], BF16, name="xTa_sb")
            nc.scalar.dma_start(xTa_sb[:], xTa)
            xTb_sb = pp.tile([P, DT, HJ], BF16, name="xTb_sb")
            nc.gpsimd.dma_start(xTb_sb[:], xTb)
            xna_sb = pp.tile([P, HT, D], BF16, name="xna_sb")
            nc.scalar.dma_start(xna_sb[:], xna)
            xnb_sb = pp.tile([P, HT, D], BF16, name="xnb_sb")
            nc.gpsimd.dma_start(xnb_sb[:], xnb)
            # late weights reuse the early-weight buffers (same byte size)
            wv_sb = pp.tile([P, DT, HW], BF16, name="wv_sb", tag="wq_sb")
            nc.scalar.dma_start(wv_sb[:], wv)
            wo_sb = pp.tile([P, HPC, D], BF16, name="wo_sb", tag="wkT_sb")
            nc.gpsimd.dma_start(wo_sb[:], wo)

            # ---- A: q = xlast @ Wq[:, hs]  ([1, 512]) -------------------
            ps_q = psB.tile([1, HW], FP32, name="ps_q", tag="psB")
            for t in range(DT):
                nc.tensor.matmul(
                    ps_q[:],
                    lhsT=xlastT_sb[:, t:t + 1],
                    rhs=wq_sb[:, t, :],
                    start=(t == 0),
                    stop=(t == DT - 1),
                )
            q_sb = wp.tile([1, HW], BF16, name="q_sb")
            nc.vector.tensor_copy(q_sb[:], ps_q[:])
            qT_sb = wp.tile([P, HPC], BF16, name="qT_sb")
            for es in range(HPC):
                ps_qt = psB.tile([P, 1], BF16, name="ps_qt", tag="psB")
                nc.tensor.transpose(
                    ps_qt[:], q_sb[:, es * P:(es + 1) * P], ident_sb[:1, :1]
                )
                nc.vector.tensor_copy(qT_sb[:, es:es + 1], ps_qt[:])
            qtil_sb = wp.tile([P, HPC, HPC], BF16, name="qtil_sb")
            nc.vector.memset(qtil_sb[:], 0.0)
            for es in range(HPC):
                nc.vector.tensor_copy(
                    qtil_sb[:, es, es:es + 1], qT_sb[:, es:es + 1])

            # ---- B: u[h, d] = sum_e Wk[d, hs+e] q[hs+e] -----------------
            u_dr = dp.tile([NH, D], BF16, name="u_dr")
            u_sb = wp.tile([HPC, D], BF16, name="u_sb", tag="udw")
            for oc in range(NJC):
                ps_u = psB.tile([HPC, JC], FP32, name="ps_u", tag="psB")
                for es in range(HPC):
                    nc.tensor.matmul(
                        ps_u[:],
                        lhsT=qtil_sb[:, es, :],
                        rhs=wkT_sb[:, es, oc * JC:(oc + 1) * JC],
                        start=(es == 0),
                        stop=(es == HPC - 1),
                    )
                nc.vector.tensor_copy(u_sb[:, oc * JC:(oc + 1) * JC], ps_u[:])
                nc.sync.dma_start(
                    u_dr[0:HPC, oc * JC:(oc + 1) * JC],
                    u_sb[:, oc * JC:(oc + 1) * JC])
            uT_sb = wp.tile([P, DT, NH], BF16, name="uT_sb")
            nc.sync.dma_start_transpose(uT_sb[:], u_dr[:])

            # ---- C: scores, pipelined per j-half ------------------------
            # sc[h, j] = sum_d u[h, d] x[j, d]; store chunks to DRAM as
            # they finish, transpose-load + exp per half.
            sc_dr = dp.tile([NH, S], BF16, name="sc_dr")
            sc_sb = wp.tile([HPC, S], BF16, name="sc_sb", tag="udw")
            eT_sb = wp.tile([P, JT, NH], BF16, name="eT_sb")
            ej_sb = wp.tile([HPC, S], BF16, name="ej_sb")
            z2_sb = wp.tile([HPC, 2], FP32, name="z2_sb")
            xT_half = [xTa_sb, xTb_sb]
            for jc in range(NJC):
                ps_s = psA.tile([HPC, JC], FP32, name="ps_s", tag="psA")
                xTh = xT_half[jc // 2]
                base = (jc % 2) * JC
                for t in range(DT):
                    nc.tensor.matmul(
                        ps_s[:],
                        lhsT=uT_sb[:, t, 0:HPC],
                        rhs=xTh[:, t, base:base + JC],
                        start=(t == 0),
                        stop=(t == DT - 1),
                    )
                nc.vector.tensor_copy(sc_sb[:, jc * JC:(jc + 1) * JC], ps_s[:])
                nc.sync.dma_start(
                    sc_dr[0:HPC, jc * JC:(jc + 1) * JC],
                    sc_sb[:, jc * JC:(jc + 1) * JC])
                if jc % 2 == 1:
                    h = jc // 2
                    nc.sync.dma_start_transpose(
                        eT_sb[:, h * HT:(h + 1) * HT, :],
                        sc_dr[:, h * HJ:(h + 1) * HJ])
                    nc.scalar.activation(
                        eT_sb[:, h * HT:(h + 1) * HT, 0:HPC],
                        eT_sb[:, h * HT:(h + 1) * HT, 0:HPC],
                        mybir.ActivationFunctionType.Exp, scale=float(ISCALE),
                    )
                    # z for this half: exp-with-accum over the row layout
                    nc.scalar.activation(
                        ej_sb[:, h * HJ:(h + 1) * HJ],
                        sc_sb[:, h * HJ:(h + 1) * HJ],
                        mybir.ActivationFunctionType.Exp, scale=float(ISCALE),
                        accum_out=z2_sb[:, h:h + 1],
                    )
            rz_sb = wp.tile([HPC, 1], FP32, name="rz_sb")
            z_sb = wp.tile([HPC, 1], FP32, name="z_sb")
            nc.vector.tensor_tensor(
                z_sb[:], z2_sb[:, 0:1], z2_sb[:, 1:2], mybir.AluOpType.add)
            nc.vector.reciprocal(rz_sb[:], z_sb[:])

            # ---- D: w_un accumulated per j-half across 4 d-chunks -------
            w_dr = dp.tile([NH, D], BF16, name="w_dr")
            w_sb = wp.tile([HPC, D], BF16, name="w_sb", tag="udw")
            xn_half = [xna_sb, xnb_sb]
            ps_w = [psW.tile([HPC, JC], FP32, name=f"ps_w{oc}", tag=f"psW{oc}")
                    for oc in range(NJC)]
            wT_sb = wp.tile([P, DT, NH], BF16, name="wT_sb")
            for h in range(2):
                xnh = xn_half[h]
                for oc in range(NJC):
                    for jt in range(HT):
                        nc.tensor.matmul(
                            ps_w[oc][:],
                            lhsT=eT_sb[:, h * HT + jt, 0:HPC],
                            rhs=xnh[:, jt, oc * JC:(oc + 1) * JC],
                            start=(h == 0 and jt == 0),
                            stop=(h == 1 and jt == HT - 1),
                        )
            for oc in range(NJC):
                if oc % 2 == 0:
                    nc.vector.tensor_scalar_mul(
                        w_sb[:, oc * JC:(oc + 1) * JC], ps_w[oc][:], rz_sb[:])
                else:
                    nc.scalar.activation(
                        w_sb[:, oc * JC:(oc + 1) * JC], ps_w[oc][:],
                        mybir.ActivationFunctionType.Copy, scale=rz_sb[:],
                    )
                nc.sync.dma_start(
                    w_dr[0:HPC, oc * JC:(oc + 1) * JC],
                    w_sb[:, oc * JC:(oc + 1) * JC])
                if oc % 2 == 1:
                    dh = oc // 2
                    nc.sync.dma_start_transpose(
                        wT_sb[:, dh * HT:(dh + 1) * HT, :],
                        w_dr[:, dh * HJ:(dh + 1) * HJ])

            # ---- E: ctx full [h, c'] accumulated per d-half -------------
            cf_sb = wp.tile([HPC, HW], BF16, name="cf_sb")
            ps_cf = psA.tile([HPC, HW], FP32, name="ps_cf", tag="psA")
            for t in range(DT):
                nc.tensor.matmul(
                    ps_cf[:],
                    lhsT=wT_sb[:, t, 0:HPC],
                    rhs=wv_sb[:, t, :],
                    start=(t == 0),
                    stop=(t == DT - 1),
                )
            nc.vector.tensor_copy(cf_sb[:], ps_cf[:])
            ctxT_sb = wp.tile([P, HPC, 1], BF16, name="ctxT_sb")
            for h in range(HPC):
                ps_ct = psB.tile([P, HPC], BF16, name="ps_ct", tag="psB")
                nc.tensor.transpose(
                    ps_ct[:], cf_sb[:, h * HD:(h + 1) * HD], ident_sb[:]
                )
                nc.vector.tensor_copy(ctxT_sb[:, h, :], ps_ct[:, h:h + 1])

            # ---- F: out partial = ctx_vec @ Wo[hs, :] + bo/4 ------------
            o_sb = wp.tile([1, D], FP32, name="o_sb")
            for oc in range(NJC):
                ps_o = psA.tile([1, JC], FP32, name="ps_o", tag="psA")
                for sub in range(HPC):
                    nc.tensor.matmul(
                        ps_o[:],
                        lhsT=ctxT_sb[:, sub, :],
                        rhs=wo_sb[:, sub, oc * JC:(oc + 1) * JC],
                        start=(sub == 0),
                        stop=(sub == HPC - 1),
                    )
                nc.vector.tensor_tensor(
                    o_sb[:, oc * JC:(oc + 1) * JC], ps_o[:],
                    bo_sb[:, oc * JC:(oc + 1) * JC], mybir.AluOpType.add,
                )
            nc.sync.dma_start(out_sh[:], o_sb[:])

    nc.compile()
    return nc


_PROGRAM = None


def _get_program():
    global _PROGRAM
    if _PROGRAM is None:
        _PROGRAM = _build_program()
    return _PROGRAM


def _shard_inputs(x, Wq, Wk, Wv, Wo, bo):
    xb = x.astype(bfloat16)
    wqb = Wq.astype(bfloat16)
    wkb = Wk.astype(bfloat16)
    wvb = Wv.astype(bfloat16)
    wob = Wo.astype(bfloat16)
    bo4 = (bo / HPC).astype(np.float32)
    identity = np.eye(HPC, dtype=bfloat16)

    in_maps = []
    for core in range(NC):
        b = core // HPC
        hg = core % HPC
        hs = slice(hg * HW, (hg + 1) * HW)
        xlastT_pre = np.ascontiguousarray(xb[b, -1, :].reshape(DT, P).T)
        wq_pre = np.ascontiguousarray(
            wqb[:, hs].reshape(DT, P, HW).transpose(1, 0, 2))
        wkT_pre = np.ascontiguousarray(
            wkb[:, hs].T.reshape(HPC, P, D).transpose(1, 0, 2))
        # xT[p, t, j] = x[b, j, t*128+p], split on j halves
        xT_pre = xb[b].T.reshape(DT, P, S).transpose(1, 0, 2)
        xTa_pre = np.ascontiguousarray(xT_pre[:, :, 0:HJ])
        xTb_pre = np.ascontiguousarray(xT_pre[:, :, HJ:S])
        # xn[p, t, d] = x[b, t*128+p, d], split on t halves
        xn_pre = xb[b].reshape(JT, P, D).transpose(1, 0, 2)
        xna_pre = np.ascontiguousarray(xn_pre[:, 0:HT, :])
        xnb_pre = np.ascontiguousarray(xn_pre[:, HT:JT, :])
        wv_pre = np.ascontiguousarray(
            wvb[:, hs].reshape(DT, P, HW).transpose(1, 0, 2))
        wo_pre = np.ascontiguousarray(
            wob[hs, :].reshape(HPC, P, D).transpose(1, 0, 2))
        in_maps.append({
            "xlastT": xlastT_pre,
            "ident": identity,
            "wq": wq_pre,
            "wkT": wkT_pre,
            "xTa": xTa_pre,
            "xTb": xTb_pre,
            "xna": xna_pre,
            "xnb": xnb_pre,
            "wv": wv_pre,
            "wo": wo_pre,
            "bo_sh": bo4,
        })
    return in_maps


def kernel(x, Wq, Wk, Wv, Wo, bo, _trace=False, _trace_cores=None):
    x = np.asarray(x, dtype=np.float32)
    Wq = np.asarray(Wq, dtype=np.float32)
    Wk = np.asarray(Wk, dtype=np.float32)
    Wv = np.asarray(Wv, dtype=np.float32)
    Wo = np.asarray(Wo, dtype=np.float32)
    bo = np.asarray(bo, dtype=np.float32)

    nc = _get_program()
    in_maps = _shard_inputs(x, Wq, Wk, Wv, Wo, bo)
    res = run_bass_kernel_spmd(
        nc, in_maps, core_ids=list(range(NC)),
        trace=_trace, trace_cores=_trace_cores,
    )
    out = np.zeros((B, D), dtype=np.float32)
    for core in range(NC):
        out[core // HPC] += res.results[core]["out_sh"][0]
    if _trace:
        kernel._last_results = res
    return out


# revision 33
# speedup vs baseline: 1.2803x; 1.2803x over previous
"""Trainium2 Bass kernel for nn_MultiHeadAttention_77232101917088.

Causal MHA where only the LAST token's projected output is returned:
    out = (softmax_causal(q k^T / sqrt(hd)) v)[:, -1, :] @ Wo + bo

Only the last query row survives, so the problem collapses (the last
causal row attends to every position):
    q[b,:]        = x[b,-1,:] @ Wq
    u[b,h,d]      = sum_e Wk[d, h*128+e] * q[b, h*128+e]
    scores[b,h,j] = sum_d x[b,j,d] * u[b,h,d]           (no K/V materialized)
    p             = softmax_j(scores * 1/sqrt(hd))
    w[b,h,d]      = sum_j p[b,h,j] * x[b,j,d]
    ctx[b, hs]    = w[b,h,:] @ Wv[:, hs]
    out           = ctx @ Wo + bo

Sharding: ZERO collectives (first-collective init costs ~74us wall on
this stack).  Each core owns one batch and 4 heads (b = core//4, head
group = core%4), computing its 4 (b,h) pairs end to end from full-depth
x[b] in both layouts; the host sums the 4 output partials per batch.

Schedule: ZERO mid-kernel DMAs — all small transposes (u, scores, w,
ctx) run on the PE in transpose mode, so nothing ever waits on the 8
shared HWDGE completion lanes (DMA bounces were measured stalling
15-35us behind unrelated bulk-load lane reuse).  The 24MB of inputs
stream on two deep rings (scalar HWDGE / gpsimd SWDGE) ordered
[weights, xT half, xn quarters, late weight] so each pipeline stage's
data lands just in time; w accumulates per xn quarter as it arrives.
The softmax z comes from exp-with-accum_out on the scalar engine;
1/sqrt(hd) is folded into exp's scale; max-subtraction is skipped
(|scores*ISCALE| < ~5 for this input class).  All data is bf16.
"""

import numpy as np
from ml_dtypes import bfloat16

import concourse.bacc as bacc
import concourse.bass as bass
import concourse.mybir as mybir
import concourse.tile as tile
from concourse.bass_utils import run_bass_kernel_spmd

P = 128          # partitions
B = 2            # batch
S = 2048         # sequence length
D = 2048         # model dim
NH = 16          # heads
HD = 128         # head dim
NC = 8           # cores
HPC = 4          # heads per core
HW = HPC * HD    # per-core head-column width (512)
DT = D // P      # depth subtiles (16)
JT = S // P      # sequence subtiles (16)
NJC = 4          # 512-wide chunks for streaming matmuls
JC = S // NJC    # 512
HJ = S // 2      # j-half width (1024)
QT = JT // 4     # subtiles per xn quarter (4)
ISCALE = 1.0 / np.sqrt(HD)

FP32 = mybir.dt.float32
BF16 = mybir.dt.bfloat16


def _build_program():
    nc = bacc.Bacc(
        "TRN2",
        target_bir_lowering=False,
        debug=False,
        enable_asserts=False,
        num_devices=NC,
    )

    # ---- per-core DRAM inputs (host pre-arranged, contiguous loads) ------
    xlastT = nc.dram_tensor("xlastT", [P, DT], BF16, kind="ExternalInput").ap()
    ident = nc.dram_tensor("ident", [HPC, HPC], BF16, kind="ExternalInput").ap()
    wq = nc.dram_tensor("wq", [P, DT, HW], BF16, kind="ExternalInput").ap()
    wkT = nc.dram_tensor("wkT", [P, HPC, D], BF16, kind="ExternalInput").ap()
    xTa = nc.dram_tensor("xTa", [P, DT, HJ], BF16, kind="ExternalInput").ap()
    xTb = nc.dram_tensor("xTb", [P, DT, HJ], BF16, kind="ExternalInput").ap()
    xnq = [nc.dram_tensor(f"xnq{i}", [P, QT, D], BF16, kind="ExternalInput").ap()
           for i in range(4)]
    wv = nc.dram_tensor("wv", [P, DT, HW], BF16, kind="ExternalInput").ap()
    wo = nc.dram_tensor("wo", [P, HPC, D], BF16, kind="ExternalInput").ap()
    bo_sh = nc.dram_tensor("bo_sh", [D], FP32, kind="ExternalInput").ap()

    out_sh = nc.dram_tensor("out_sh", [1, D], FP32, kind="ExternalOutput").ap()

    with tile.TileContext(nc) as tc:
        with (
            tc.tile_pool(name="persist", bufs=1) as pp,
            tc.tile_pool(name="work", bufs=1) as wp,
            tc.tile_pool(name="psA", bufs=2, space="PSUM") as psA,
            tc.tile_pool(name="psW", bufs=1, space="PSUM") as psW,
            tc.tile_pool(name="psB", bufs=2, space="PSUM") as psB,
        ):
            # ---- loads: two deep bulk rings, sync only tiny + out -------
            xlastT_sb = pp.tile([P, DT], BF16, name="xlastT_sb")
            nc.sync.dma_start(xlastT_sb[:], xlastT)
            ident_sb = pp.tile([HPC, HPC], BF16, name="ident_sb")
            nc.sync.dma_start(ident_sb[:], ident)
            bo_sb = pp.tile([1, D], FP32, name="bo_sb")
            nc.sync.dma_start(bo_sb[:], bo_sh.rearrange("(o m) -> o m", o=1))

            wq_sb = pp.tile([P, DT, HW], BF16, name="wq_sb")
            nc.scalar.dma_start(wq_sb[:], wq)
            wkT_sb = pp.tile([P, HPC, D], BF16, name="wkT_sb")
            nc.gpsimd.dma_start(wkT_sb[:], wkT)
            xTa_sb = pp.tile([P, DT, HJ], BF16, name="xTa_sb")
            nc.scalar.dma_start(xTa_sb[:], xTa)
            xTb_sb = pp.tile([P, DT, HJ], BF16, name="xTb_sb")
            nc.gpsimd.dma_start(xTb_sb[:], xTb)
            xnq_sb = [pp.tile([P, QT, D], BF16, name=f"xnq_sb{i}")
                      for i in range(4)]
            nc.scalar.dma_start(xnq_sb[0][:], xnq[0])
            nc.gpsimd.dma_start(xnq_sb[1][:], xnq[1])
            nc.scalar.dma_start(xnq_sb[2][:], xnq[2])
            nc.gpsimd.dma_start(xnq_sb[3][:], xnq[3])
            wv_sb = pp.tile([P, DT, HW], BF16, name="wv_sb", tag="wq_sb")
            nc.scalar.dma_start(wv_sb[:], wv)
            wo_sb = pp.tile([P, HPC, D], BF16, name="wo_sb", tag="wkT_sb")
            nc.gpsimd.dma_start(wo_sb[:], wo)

            # ---- A: q = xlast @ Wq[:, hs]  ([1, 512]) -------------------
            ps_q = psB.tile([1, HW], FP32, name="ps_q", tag="psB")
            for t in range(DT):
                nc.tensor.matmul(
                    ps_q[:],
                    lhsT=xlastT_sb[:, t:t + 1],
                    rhs=wq_sb[:, t, :],
                    start=(t == 0),
                    stop=(t == DT - 1),
                )
            q_sb = wp.tile([1, HW], BF16, name="q_sb")
            nc.vector.tensor_copy(q_sb[:], ps_q[:])
            qT_sb = wp.tile([P, HPC], BF16, name="qT_sb")
            for es in range(HPC):
                ps_qt = psB.tile([P, 1], BF16, name="ps_qt", tag="psB")
                nc.tensor.transpose(
                    ps_qt[:], q_sb[:, es * P:(es + 1) * P], ident_sb[:1, :1]
                )
                nc.vector.tensor_copy(qT_sb[:, es:es + 1], ps_qt[:])
            qtil_sb = wp.tile([P, HPC, HPC], BF16, name="qtil_sb")
            nc.vector.memset(qtil_sb[:], 0.0)
            for es in range(HPC):
                nc.vector.tensor_copy(
                    qtil_sb[:, es, es:es + 1], qT_sb[:, es:es + 1])

            # ---- B: u[h, d], then PE-transpose to uT[p, t, h] -----------
            u_sb = wp.tile([HPC, D], BF16, name="u_sb", tag="udw")
            for oc in range(NJC):
                ps_u = psB.tile([HPC, JC], FP32, name="ps_u", tag="psB")
                for es in range(HPC):
                    nc.tensor.matmul(
                        ps_u[:],
                        lhsT=qtil_sb[:, es, :],
                        rhs=wkT_sb[:, es, oc * JC:(oc + 1) * JC],
                        start=(es == 0),
                        stop=(es == HPC - 1),
                    )
                nc.vector.tensor_copy(u_sb[:, oc * JC:(oc + 1) * JC], ps_u[:])
            uT_sb = wp.tile([P, DT, HPC], BF16, name="uT_sb")
            for t in range(DT):
                ps_t = psB.tile([P, HPC], BF16, name="ps_ut", tag="psB")
                nc.tensor.transpose(
                    ps_t[:], u_sb[:, t * P:(t + 1) * P], ident_sb[:]
                )
                nc.vector.tensor_copy(uT_sb[:, t, :], ps_t[:])

            # ---- C: scores per j-half; PE-transpose + exp ---------------
            sc_sb = wp.tile([HPC, S], BF16, name="sc_sb", tag="udw")
            eT_sb = wp.tile([P, JT, HPC], BF16, name="eT_sb")
            ej_sb = wp.tile([HPC, S], BF16, name="ej_sb")
            z2_sb = wp.tile([HPC, 2], FP32, name="z2_sb")
            xT_half = [xTa_sb, xTb_sb]
            for jc in range(NJC):
                ps_s = psA.tile([HPC, JC], FP32, name="ps_s", tag="psA")
                xTh = xT_half[jc // 2]
                base = (jc % 2) * JC
                for t in range(DT):
                    nc.tensor.matmul(
                        ps_s[:],
                        lhsT=uT_sb[:, t, :],
                        rhs=xTh[:, t, base:base + JC],
                        start=(t == 0),
                        stop=(t == DT - 1),
                    )
                nc.vector.tensor_copy(sc_sb[:, jc * JC:(jc + 1) * JC], ps_s[:])
                if jc % 2 == 1:
                    h = jc // 2
                    for lt in range(JT // 2):
                        t = h * (JT // 2) + lt
                        ps_e = psB.tile([P, HPC], BF16, name="ps_e", tag="psB")
                        nc.tensor.transpose(
                            ps_e[:], sc_sb[:, t * P:(t + 1) * P], ident_sb[:]
                        )
                        nc.vector.tensor_copy(eT_sb[:, t, :], ps_e[:])
                    nc.scalar.activation(
                        eT_sb[:, h * (JT // 2):(h + 1) * (JT // 2), :],
                        eT_sb[:, h * (JT // 2):(h + 1) * (JT // 2), :],
                        mybir.ActivationFunctionType.Exp, scale=float(ISCALE),
                    )
                    # softmax denominator for this half via exp+accum
                    nc.scalar.activation(
                        ej_sb[:, h * HJ:(h + 1) * HJ],
                        sc_sb[:, h * HJ:(h + 1) * HJ],
                        mybir.ActivationFunctionType.Exp, scale=float(ISCALE),
                        accum_out=z2_sb[:, h:h + 1],
                    )
            rz_sb = wp.tile([HPC, 1], FP32, name="rz_sb")
            z_sb = wp.tile([HPC, 1], FP32, name="z_sb")
            nc.vector.tensor_tensor(
                z_sb[:], z2_sb[:, 0:1], z2_sb[:, 1:2], mybir.AluOpType.add)
            nc.vector.reciprocal(rz_sb[:], z_sb[:])

            # ---- D: w_un accumulated per xn quarter ---------------------
            w_sb = wp.tile([HPC, D], BF16, name="w_sb", tag="udw")
            ps_w = [psW.tile([HPC, JC], FP32, name=f"ps_w{oc}", tag=f"psW{oc}")
                    for oc in range(NJC)]
            for qi in range(4):
                for oc in range(NJC):
                    for jt in range(QT):
                        nc.tensor.matmul(
                            ps_w[oc][:],
                            lhsT=eT_sb[:, qi * QT + jt, :],
                            rhs=xnq_sb[qi][:, jt, oc * JC:(oc + 1) * JC],
                            start=(qi == 0 and jt == 0),
                            stop=(qi == 3 and jt == QT - 1),
                        )
            for oc in range(NJC):
                if oc % 2 == 0:
                    nc.vector.tensor_scalar_mul(
                        w_sb[:, oc * JC:(oc + 1) * JC], ps_w[oc][:], rz_sb[:])
                else:
                    nc.scalar.activation(
                        w_sb[:, oc * JC:(oc + 1) * JC], ps_w[oc][:],
                        mybir.ActivationFunctionType.Copy, scale=rz_sb[:],
                    )
            wT_sb = wp.tile([P, DT, HPC], BF16, name="wT_sb")
            for t in range(DT):
                ps_t = psB.tile([P, HPC], BF16, name="ps_wt", tag="psB")
                nc.tensor.transpose(
                    ps_t[:], w_sb[:, t * P:(t + 1) * P], ident_sb[:]
                )
                nc.vector.tensor_copy(wT_sb[:, t, :], ps_t[:])

            # ---- E: ctx full [h, c'], keep per-head diagonal blocks -----
            cf_sb = wp.tile([HPC, HW], BF16, name="cf_sb")
            ps_cf = psA.tile([HPC, HW], FP32, name="ps_cf", tag="psA")
            for t in range(DT):
                nc.tensor.matmul(
                    ps_cf[:],
                    lhsT=wT_sb[:, t, :],
                    rhs=wv_sb[:, t, :],
                    start=(t == 0),
                    stop=(t == DT - 1),
                )
            nc.vector.tensor_copy(cf_sb[:], ps_cf[:])
            ctxT_sb = wp.tile([P, HPC, 1], BF16, name="ctxT_sb")
            for h in range(HPC):
                ps_ct = psB.tile([P, HPC], BF16, name="ps_ct", tag="psB")
                nc.tensor.transpose(
                    ps_ct[:], cf_sb[:, h * HD:(h + 1) * HD], ident_sb[:]
                )
                nc.vector.tensor_copy(ctxT_sb[:, h, :], ps_ct[:, h:h + 1])

            # ---- F: out partial = ctx_vec @ Wo[hs, :] + bo/4 ------------
            o_sb = wp.tile([1, D], FP32, name="o_sb")
            for oc in range(NJC):
                ps_o = psA.tile([1, JC], FP32, name="ps_o", tag="psA")
                for sub in range(HPC):
                    nc.tensor.matmul(
                        ps_o[:],
                        lhsT=ctxT_sb[:, sub, :],
                        rhs=wo_sb[:, sub, oc * JC:(oc + 1) * JC],
                        start=(sub == 0),
                        stop=(sub == HPC - 1),
                    )
                nc.vector.tensor_tensor(
                    o_sb[:, oc * JC:(oc + 1) * JC], ps_o[:],
                    bo_sb[:, oc * JC:(oc + 1) * JC], mybir.AluOpType.add,
                )
            nc.sync.dma_start(out_sh[:], o_sb[:])

    nc.compile()
    return nc


_PROGRAM = None


def _get_program():
    global _PROGRAM
    if _PROGRAM is None:
        _PROGRAM = _build_program()
    return _PROGRAM


def _shard_inputs(x, Wq, Wk, Wv, Wo, bo):
    xb = x.astype(bfloat16)
    wqb = Wq.astype(bfloat16)
    wkb = Wk.astype(bfloat16)
    wvb = Wv.astype(bfloat16)
    wob = Wo.astype(bfloat16)
    bo4 = (bo / HPC).astype(np.float32)
    identity = np.eye(HPC, dtype=bfloat16)

    in_maps = []
    for core in range(NC):
        b = core // HPC
        hg = core % HPC
        hs = slice(hg * HW, (hg + 1) * HW)
        xlastT_pre = np.ascontiguousarray(xb[b, -1, :].reshape(DT, P).T)
        wq_pre = np.ascontiguousarray(
            wqb[:, hs].reshape(DT, P, HW).transpose(1, 0, 2))
        wkT_pre = np.ascontiguousarray(
            wkb[:, hs].T.reshape(HPC, P, D).transpose(1, 0, 2))
        xT_pre = xb[b].T.reshape(DT, P, S).transpose(1, 0, 2)
        xTa_pre = np.ascontiguousarray(xT_pre[:, :, 0:HJ])
        xTb_pre = np.ascontiguousarray(xT_pre[:, :, HJ:S])
        xn_pre = xb[b].reshape(JT, P, D).transpose(1, 0, 2)
        xnq_pre = [np.ascontiguousarray(xn_pre[:, i * QT:(i + 1) * QT, :])
                   for i in range(4)]
        wv_pre = np.ascontiguousarray(
            wvb[:, hs].reshape(DT, P, HW).transpose(1, 0, 2))
        wo_pre = np.ascontiguousarray(
            wob[hs, :].reshape(HPC, P, D).transpose(1, 0, 2))
        m = {
            "xlastT": xlastT_pre,
            "ident": identity,
            "wq": wq_pre,
            "wkT": wkT_pre,
            "xTa": xTa_pre,
            "xTb": xTb_pre,
            "wv": wv_pre,
            "wo": wo_pre,
            "bo_sh": bo4,
        }
        for i in range(4):
            m[f"xnq{i}"] = xnq_pre[i]
        in_maps.append(m)
    return in_maps


def kernel(x, Wq, Wk, Wv, Wo, bo, _trace=False, _trace_cores=None):
    x = np.asarray(x, dtype=np.float32)
    Wq = np.asarray(Wq, dtype=np.float32)
    Wk = np.asarray(Wk, dtype=np.float32)
    Wv = np.asarray(Wv, dtype=np.float32)
    Wo = np.asarray(Wo, dtype=np.float32)
    bo = np.asarray(bo, dtype=np.float32)

    nc = _get_program()
    in_maps = _shard_inputs(x, Wq, Wk, Wv, Wo, bo)
    res = run_bass_kernel_spmd(
        nc, in_maps, core_ids=list(range(NC)),
        trace=_trace, trace_cores=_trace_cores,
    )
    out = np.zeros((B, D), dtype=np.float32)
    for core in range(NC):
        out[core // HPC] += res.results[core]["out_sh"][0]
    if _trace:
        kernel._last_results = res
    return out
